# revision 4
# baseline (speedup 1.0000x reference)
"""Chunked sliding-window attention (B=2, T=8192, H=16, Dh=128, W=256) on 8
Trainium2 NeuronCores.

Sharding: 8 cores = 2 (batch) x 4 (head groups of 4 heads). Each core computes
q/k/v projections for its 512-wide slice of the 2048 projection dims, RoPE,
chunked attention for its 4 heads, and a partial output projection over its
512 rows of Wo^T. The host sums the 4 partial outputs per batch element.

Device layouts (host-prepared):
  xt   [128, 16, T]   x^T tiles: xt[p, kt, t] = x[b, t, kt*128+p]        (bf16)
  xt8  [128, KA, T]   16*x^T tiles for kt<KA                             (e4m3)
  wq/wk[128, HS, KT-KA, 128] 1024*(Wq_perm)^T bf16 k-tiles >= KA         (bf16)
  wq8/wk8[128, HS, KA, 128]  64*(Wq_perm)^T fp8 k-tiles < KA             (e4m3)
  wv   [128, 16, 512] Wv^T slice (unpermuted)                            (bf16)
  wo   [128, 4, 2048] Wo^T rows for this core's 512 dims                 (bf16)
  ccat [128, T]       [cos; cos] rope table (freq idx on partitions)     (bf16)
  scat [128, T]       [-sin; sin]                                        (bf16)
  mask [128, 2, 256]  transposed causal 0/1 masks for own-chunk kv tiles (bf16)

The rope row-permutation maps interleaved (re,im) pairs to split layout
(re block rows 0..63, im rows 64..127 per head); applied identically to q and
k it leaves scores invariant, and makes rope unit-stride on chip.

Attention is computed in transposed-score layout [kv, q]: masking is a 0/1
multiply after exp; the softmax denominator (a cross-partition sum of the
pre-summed exp tiles) runs on GPSIMD partition_all_reduce, off the PE.

Mixed precision: KA of the 16 contraction tiles of the q/k projections run
as plain e4m3 DoubleRow matmuls (2 k-tiles per pass, ~1.9x the bf16 rate).
Scale frames are uniform powers of two - fp8 x at 16x, fp8 W at 64x, bf16 W
at 1024x - so fp8 and bf16 products accumulate in one PSUM group at 1024x,
folded back in the exp scale. Measured end-to-end rel_max stays ~1.3e-2 at
KA=6 (error scales as sqrt(KA/16) of the all-fp8 4.6e-2). v/o projections
and attention stay bf16: each would add its own quadrature error term and
v feeds the output directly.

Performance notes (measured via NTFF traces; PE busy ~96%):
- PSUM (8 banks) fully budgeted: 2 banks q/k/v proj, 2 o-proj, 2 packed
  score pairs, 2 ou tiles. o-projection is deferred one block so it never
  heads the PE queue while wo / next xt are still in flight.
- The softmax denominator pre-sums the 4 exp tiles on DVE, then ONE gpsimd
  partition_all_reduce per (head, chunk) replaces the ones-matmul (PE is
  the bottleneck; rc is consumed a block later so the ~1.7us PAR latency
  hides).
- Own-chunk kv tile 1 is causally dead for q cols 0..127: scores, exp and
  the AV matmul all run on the live half only (AV order 0,1,3,2 keeps the
  accumulation group's stop flag on a full-width matmul).
- Const DMAs: wq/wk/xt stream on the sync queue in first-PE-use order;
  wv/mask/wo stream concurrently on the scalar engine's DMA queue.
- fp8 everywhere was evaluated and rejected: e4m3 projections give
  rel_max ~4.6e-2 vs the 2e-2 budget; per-projection plain fp8 is 2.2e-2+.
  3-term hi/lo splits are accurate but slower than bf16 on real HW
  (DoubleRow measures 1.9x bf16, so 3 split terms cost 1.5x).
"""

import os

import numpy as np
import ml_dtypes

N_HEAD = 16
HEAD_DIM = 128
WINDOW = 256
THETA = 10000.0
B = 2
T = 8192
DM = 2048
KT = DM // 128      # 16 contraction tiles
KA = 6              # contraction tiles in fp8 for q/k projections (even)
HS = 4              # heads per core
DS = HS * HEAD_DIM  # 512 projection dims per core
BLK = 512           # tokens per pipeline block (2 chunks)
CH = WINDOW         # 256
SCALE = float(HEAD_DIM) ** -0.5
SX8, SW8, SWB = 16.0, 64.0, 1024.0  # fp8 x / fp8 W / bf16 W scale frames

LAST_EXEC_NS = None
_NC = None

bf16 = ml_dtypes.bfloat16
f8 = ml_dtypes.float8_e4m3


def _build_nc(t_len=T):
    from contextlib import ExitStack

    import concourse.tile as tile
    from concourse import bacc, mybir
    from concourse.bass_isa import ReduceOp

    fp32 = mybir.dt.float32
    b16 = mybir.dt.bfloat16
    e4 = mybir.dt.float8e4
    DR = mybir.MatmulPerfMode.DoubleRow

    nb = t_len // BLK
    nc = bacc.Bacc("TRN2", target_bir_lowering=False, debug=False)

    KB = KT - KA  # bf16 k-tiles
    xt = nc.dram_tensor(
        "xt", [nb, 128, KT, BLK], b16, kind="ExternalInput"
    ).ap()
    xt8 = nc.dram_tensor(
        "xt8", [nb, 128, max(KA, 1), BLK], e4, kind="ExternalInput"
    ).ap()
    # head-major so the prologue can stream exactly the head the PE needs next
    wq = nc.dram_tensor("wq", [128, HS, KB, 128], b16, kind="ExternalInput").ap()
    wk = nc.dram_tensor("wk", [128, HS, KB, 128], b16, kind="ExternalInput").ap()
    wq8 = nc.dram_tensor("wq8", [128, HS, max(KA, 1), 128], e4, kind="ExternalInput").ap()
    wk8 = nc.dram_tensor("wk8", [128, HS, max(KA, 1), 128], e4, kind="ExternalInput").ap()
    wv = nc.dram_tensor("wv", [128, KT, DS], b16, kind="ExternalInput").ap()
    wo = nc.dram_tensor("wo", [128, HS, DM], b16, kind="ExternalInput").ap()
    ccat = nc.dram_tensor("ccat", [128, t_len], b16, kind="ExternalInput").ap()
    scat = nc.dram_tensor("scat", [128, t_len], b16, kind="ExternalInput").ap()
    mask = nc.dram_tensor("mask", [128, 2, CH], b16, kind="ExternalInput").ap()
    y = nc.dram_tensor("y", [t_len, DM], fp32, kind="ExternalOutput").ap()

    Exp = mybir.ActivationFunctionType.Exp
    ESCALE = SCALE / float(SX8 * SW8) ** 2  # scores carry (16*64)^2

    with tile.TileContext(nc) as tc, ExitStack() as ctx:
        const = ctx.enter_context(tc.tile_pool(name="const", bufs=1))
        xt_p = ctx.enter_context(tc.tile_pool(name="xtp", bufs=2))
        xt8_p = ctx.enter_context(tc.tile_pool(name="xt8p", bufs=2))
        raw_p = ctx.enter_context(tc.tile_pool(name="rawp", bufs=3))
        swp_p = ctx.enter_context(tc.tile_pool(name="swpp", bufs=3))
        tmp_p = ctx.enter_context(tc.tile_pool(name="tmpp", bufs=3))
        qr_p = ctx.enter_context(tc.tile_pool(name="qrp", bufs=8))
        kr_p = ctx.enter_context(tc.tile_pool(name="krp", bufs=10))
        v_p = ctx.enter_context(tc.tile_pool(name="vp", bufs=10))
        e_p = ctx.enter_context(tc.tile_pool(name="ep", bufs=12))
        dn_p = ctx.enter_context(tc.tile_pool(name="dnp", bufs=4))
        rc_p = ctx.enter_context(tc.tile_pool(name="rcp", bufs=4))
        ot_p = ctx.enter_context(tc.tile_pool(name="otp", bufs=24))
        y_p = ctx.enter_context(tc.tile_pool(name="yp", bufs=2))
        tab_p = ctx.enter_context(tc.tile_pool(name="tabp", bufs=3))
        # PSUM is 8 banks x [128,512] fp32; bufs are bank-granular.
        ps_big = ctx.enter_context(tc.tile_pool(name="psbig", bufs=2, space="PSUM"))
        ps_op = ctx.enter_context(tc.tile_pool(name="psop", bufs=2, space="PSUM"))
        ps_st = ctx.enter_context(tc.tile_pool(name="psst", bufs=2, space="PSUM"))
        ps_do = ctx.enter_context(tc.tile_pool(name="psdo", bufs=2, space="PSUM"))

        wq_sb = const.tile([128, HS, KB, 128], b16)
        wk_sb = const.tile([128, HS, KB, 128], b16)
        wq8_sb = const.tile([128, HS, max(KA, 1), 128], e4)
        wk8_sb = const.tile([128, HS, max(KA, 1), 128], e4)
        wv_sb = const.tile([128, KT, DS], b16)
        wo_sb = const.tile([128, HS, DM], b16)
        mask_sb = const.tile([128, 2, CH], b16)

        def fetch_tables(t0):
            cc = tab_p.tile([128, BLK], b16, tag="cc")
            nc.sync.dma_start(cc, ccat[:, t0 : t0 + BLK])
            sc = tab_p.tile([128, BLK], b16, tag="sc")
            nc.sync.dma_start(sc, scat[:, t0 : t0 + BLK])
            return cc, sc

        def fetch_xts(blk):
            xsb = xt_p.tile([128, KT, BLK], b16, tag="xt")
            nc.sync.dma_start(xsb, xt[blk])
            if KA > 0:
                x8 = xt8_p.tile([128, KA, BLK], e4, tag="xt8")
                nc.sync.dma_start(x8, xt8[blk][:, 0:KA, :])
            else:
                x8 = None
            return xsb, x8

        # Sync queue: pieces ordered by first PE use. q h0 needs wq8/xt8
        # first (fp8 DR matmuls lead each psum), then bf16 wq/xt tiles.
        # Scalar engine's DMA queue concurrently streams wv/mask/wo.
        xt_first = xt_p.tile([128, KT, BLK], b16, tag="xt")
        if KA > 0:
            xt8_first = xt8_p.tile([128, KA, BLK], e4, tag="xt8")
            nc.sync.dma_start(wq8_sb[:, 0], wq8[:, 0])
            nc.sync.dma_start(xt8_first, xt8[0][:, 0:KA, :])
        else:
            xt8_first = None
        nc.sync.dma_start(wq_sb[:, 0, 0:2, :], wq[:, 0, 0:2, :])
        nc.sync.dma_start(xt_first[:, KA : KA + 4, :], xt[0][:, KA : KA + 4, :])
        nc.sync.dma_start(wq_sb[:, 0, 2:KB, :], wq[:, 0, 2:KB, :])
        nc.sync.dma_start(xt_first[:, KA + 4 : KT, :], xt[0][:, KA + 4 : KT, :])
        if KA > 0:
            nc.sync.dma_start(wk8_sb[:, 0], wk8[:, 0])
        nc.sync.dma_start(wk_sb[:, 0], wk[:, 0])
        nc.scalar.dma_start(wv_sb[:, 0:8, :], wv[:, 0:8, :])
        tab_first = fetch_tables(0)
        nc.sync.dma_start(xt_first[:, 0:KA, :], xt[0][:, 0:KA, :])
        for h in range(1, HS):
            if KA > 0:
                nc.sync.dma_start(wq8_sb[:, h], wq8[:, h])
                nc.sync.dma_start(wk8_sb[:, h], wk8[:, h])
            nc.sync.dma_start(wq_sb[:, h], wq[:, h])
            nc.sync.dma_start(wk_sb[:, h], wk[:, h])
        nc.scalar.dma_start(mask_sb, mask)
        nc.scalar.dma_start(wv_sb[:, 8:KT, :], wv[:, 8:KT, :])
        for h in range(HS):
            nc.scalar.dma_start(wo_sb[:, h, :], wo[:, h, :])
        # xt2/tab2 deferred behind blk0's rope swap DMAs on the sync queue
        tab_second = [None]
        xt_second = [None]

        def emit_deferred_consts():
            if nb > 1:
                tab_second[0] = fetch_tables(BLK)
                xt_second[0] = fetch_xts(1)

        prev_k = [None] * HS
        prev_v = [None, None]
        pend_ot = None
        for blk in range(nb):
            t0 = blk * BLK
            if blk == 0:
                xt_sb, xt8_sb = xt_first, xt8_first
            elif blk == 1 and xt_second[0] is not None:
                xt_sb, xt8_sb = xt_second[0]
            else:
                xt_sb, xt8_sb = fetch_xts(blk)

            if blk == 0:
                c_sl, s_sl = tab_first
            elif blk == 1 and tab_second[0] is not None:
                c_sl, s_sl = tab_second[0]
            else:
                c_sl, s_sl = fetch_tables(t0)
            cur_q = []
            cur_k = []
            for h in range(HS):
                for w_sb, w8_sb, dst in (
                    (wq_sb, wq8_sb, cur_q),
                    (wk_sb, wk8_sb, cur_k),
                ):
                    ps = ps_big.tile([128, BLK], fp32, tag="psbig")
                    for j in range(KA // 2):
                        nc.tensor.matmul(
                            ps,
                            lhsT=w8_sb[:, h, 2 * j : 2 * j + 2, :],
                            rhs=xt8_sb[:, 2 * j : 2 * j + 2, :],
                            start=(j == 0),
                            stop=False,
                            perf_mode=DR,
                        )
                    for k in range(KB):
                        nc.tensor.matmul(
                            ps,
                            lhsT=w_sb[:, h, k, :],
                            rhs=xt_sb[:, KA + k, :],
                            start=(KA == 0 and k == 0),
                            stop=(k == KB - 1),
                        )
                    raw = raw_p.tile([128, BLK], b16, tag="raw")
                    nc.scalar.copy(raw, ps)
                    # swap the (re, im) halves via SBUF->SBUF DMA (DVE lanes
                    # cannot cross partitions)
                    swp = swp_p.tile([128, BLK], b16, tag="swp")
                    nc.sync.dma_start(swp[0:64, :], raw[64:128, :])
                    nc.sync.dma_start(swp[64:128, :], raw[0:64, :])
                    t1 = tmp_p.tile([128, BLK], b16, tag="t1")
                    nc.vector.tensor_mul(t1, raw, c_sl)
                    t2 = tmp_p.tile([128, BLK], b16, tag="t2")
                    nc.vector.tensor_mul(t2, swp, s_sl)
                    if dst is cur_q:
                        rot = qr_p.tile([128, BLK], b16, tag="qr")
                    else:
                        rot = kr_p.tile([128, BLK], b16, tag="kr")
                    nc.vector.tensor_add(rot, t1, t2)
                    dst.append(rot)

            cur_v = []
            for tt in range(4):
                ps = ps_big.tile([128, BLK], fp32, tag="psbig")
                for k in range(KT):
                    nc.tensor.matmul(
                        ps,
                        lhsT=xt_sb[:, k, tt * 128 : (tt + 1) * 128],
                        rhs=wv_sb[:, k, :],
                        start=(k == 0),
                        stop=(k == KT - 1),
                    )
                vt = v_p.tile([128, DS], b16, tag="v")
                nc.vector.tensor_copy(out=vt, in_=ps)
                cur_v.append(vt)

            ot_tiles = {}
            for ci in range(2):
                c = 2 * blk + ci
                qoff = ci * CH
                js = [2, 3] if c == 0 else [0, 1, 2, 3]
                for h in range(HS):
                    q_sl = cur_q[h][:, qoff : qoff + CH]
                    es = []
                    stp = None
                    for idx, j in enumerate(js):
                        if j < 2:
                            if ci == 1:
                                ksrc = cur_k[h][:, j * 128 : (j + 1) * 128]
                            else:
                                ksrc = prev_k[h][:, CH + j * 128 : CH + (j + 1) * 128]
                        else:
                            ksrc = cur_k[h][:, qoff + (j - 2) * 128 : qoff + (j - 1) * 128]
                        if idx % 2 == 0:
                            stp = ps_st.tile([128, 2 * CH], fp32, tag="st")
                        st = stp[:, (idx % 2) * CH : (idx % 2 + 1) * CH]
                        e = e_p.tile([128, CH], b16, tag="e")
                        if j == 3:
                            # kv rows 128.. of own chunk are causally dead for
                            # q cols 0..127: compute only the live half
                            nc.tensor.matmul(
                                st[:, 128:CH], lhsT=ksrc, rhs=q_sl[:, 128:CH],
                                start=True, stop=True,
                            )
                            nc.gpsimd.memset(e[:, 0:128], 0.0)
                            nc.scalar.activation(
                                e[:, 128:CH], st[:, 128:CH], Exp, scale=ESCALE
                            )
                            nc.vector.tensor_mul(
                                e[:, 128:CH], e[:, 128:CH], mask_sb[:, 1, 128:CH]
                            )
                        else:
                            nc.tensor.matmul(st, lhsT=ksrc, rhs=q_sl, start=True, stop=True)
                            nc.scalar.activation(e, st, Exp, scale=ESCALE)
                            if j == 2:
                                nc.vector.tensor_mul(e, e, mask_sb[:, 0, :])
                        es.append((j, e))
                    # pre-sum the exp tiles on DVE; ONE gpsimd
                    # partition_all_reduce then yields the denominator
                    # broadcast across partitions, keeping it off the PE.
                    acc = es[0][1]
                    for i in range(1, len(es)):
                        if i == len(es) - 1:
                            nxt = e_p.tile([128, CH], b16, tag="esum")
                        else:
                            nxt = tmp_p.tile([128, CH], b16, tag=f"ea{i % 2}")
                        nc.vector.tensor_add(nxt, acc, es[i][1])
                        acc = nxt
                    esum = acc
                    ou = ps_do.tile([128, CH], fp32, tag="ou")
                    # AV accumulation order 0,1,3,2: j=3 streams only its
                    # causally-live half, the full-width j=2 carries stop.
                    avs = [(i, j, e) for i, (j, e) in enumerate(es)]
                    if len(avs) == 4:
                        avs = [avs[0], avs[1], avs[3], avs[2]]
                    for pos, (i, j, e) in enumerate(avs):
                        if j < 2:
                            vsrc = cur_v[j] if ci == 1 else prev_v[j]
                        else:
                            vsrc = cur_v[2 * ci + (j - 2)]
                        vs = vsrc[:, h * 128 : (h + 1) * 128]
                        if j == 3 and pos != len(avs) - 1 and pos != 0:
                            nc.tensor.matmul(
                                ou[:, 128:CH], lhsT=vs, rhs=e[:, 128:CH],
                                start=False, stop=False,
                            )
                        else:
                            nc.tensor.matmul(
                                ou, lhsT=vs, rhs=e,
                                start=(pos == 0), stop=(pos == len(avs) - 1),
                            )
                    dn = dn_p.tile([128, CH], fp32, tag="dn")
                    nc.gpsimd.partition_all_reduce(dn, esum, 128, ReduceOp.add)
                    rc = rc_p.tile([128, CH], fp32, tag="rc")
                    nc.vector.reciprocal_approx_fast(out=rc, in_=dn)
                    ot = ot_p.tile([128, CH], b16, tag="ot")
                    nc.vector.tensor_mul(ot, ou, rc)
                    ot_tiles[(h, ci)] = ot

            # deferred const DMAs (tab2/xt2) go behind blk0's rope-swap
            # DMAs in the serial sync queue, not ahead of them
            if blk == 0:
                emit_deferred_consts()

            def emit_oproj(ot_map, base_t0, final=False):
                for tt in range(4):
                    ci, sub = tt // 2, tt % 2
                    ysb = y_p.tile([128, DM], fp32, tag="y")
                    for ct in range(4):
                        yps = ps_op.tile([128, 512], fp32, tag="psop")
                        for h in range(HS):
                            nc.tensor.matmul(
                                yps,
                                lhsT=ot_map[(h, ci)][:, sub * 128 : (sub + 1) * 128],
                                rhs=wo_sb[:, h, ct * 512 : (ct + 1) * 512],
                                start=(h == 0),
                                stop=(h == HS - 1),
                            )
                        ysl = ysb[:, ct * 512 : (ct + 1) * 512]
                        if final:
                            # tail has nothing to interleave: halve the copy
                            # latency (scalar+vector in parallel) and stream
                            # y out per-ct so the last DMA overlaps compute
                            nc.scalar.copy(ysl[:, 0:256], yps[:, 0:256])
                            nc.vector.tensor_copy(out=ysl[:, 256:512], in_=yps[:, 256:512])
                            nc.scalar.dma_start(
                                y[
                                    base_t0 + tt * 128 : base_t0 + (tt + 1) * 128,
                                    ct * 512 : (ct + 1) * 512,
                                ],
                                ysl,
                            )
                        elif ct % 2 == 0:
                            nc.scalar.copy(ysl, yps)
                        else:
                            nc.vector.tensor_copy(out=ysl, in_=yps)
                    if not final:
                        nc.scalar.dma_start(
                            y[base_t0 + tt * 128 : base_t0 + (tt + 1) * 128, :], ysb
                        )

            # o-projection deferred one block so it never heads the PE queue
            # while wo / next xt are still in flight
            if pend_ot is not None:
                emit_oproj(pend_ot[0], pend_ot[1])
            pend_ot = (ot_tiles, t0)
            if blk == nb - 1:
                emit_oproj(ot_tiles, t0, final=True)
                pend_ot = None

            prev_k = cur_k
            prev_v = cur_v[2:4]

    nc.compile()
    return nc


def _rope_perm():
    perm = np.empty(DM, np.int64)
    for h in range(N_HEAD):
        base = h * HEAD_DIM
        perm[base : base + 64] = base + 2 * np.arange(64)
        perm[base + 64 : base + 128] = base + 2 * np.arange(64) + 1
    return perm


def _prep_inputs(x, Wq, Wk, Wv, Wo, t_len=T):
    """Build per-core in_maps. Cores 0-3: batch 0, head groups 0-3; 4-7: batch 1."""
    x = np.asarray(x, dtype=np.float32)
    Wq = np.asarray(Wq, dtype=np.float32)
    Wk = np.asarray(Wk, dtype=np.float32)
    Wv = np.asarray(Wv, dtype=np.float32)
    Wo = np.asarray(Wo, dtype=np.float32)
    nb_b = x.shape[0]
    KB = KT - KA

    perm = _rope_perm()
    wqT = np.ascontiguousarray(Wq[perm].T).astype(np.float32)  # [K, dout_perm]
    wkT = np.ascontiguousarray(Wk[perm].T).astype(np.float32)
    wvT = np.ascontiguousarray(Wv.T).astype(bf16)
    woT = np.ascontiguousarray(Wo.T).astype(bf16)        # [d, c]

    # xt[blk, p, kt, t_in_blk] = x[b, blk*BLK + t, kt*128+p] - block-major so
    # each block's slab is one fully-contiguous DMA read per partition
    nblk = t_len // BLK
    xts, xt8s = [], []
    for b in range(nb_b):
        xT = x[b].T.reshape(KT, 128, nblk, BLK)
        xts.append(np.ascontiguousarray(xT.transpose(2, 1, 0, 3)).astype(bf16))
        x8 = (xT[: max(KA, 1)] * SX8).transpose(2, 1, 0, 3)
        xt8s.append(np.ascontiguousarray(x8).astype(f8))

    wq_s, wk_s, wv_s, wo_s, wq8_s, wk8_s = [], [], [], [], [], []
    for hg in range(4):
        sl = slice(hg * DS, (hg + 1) * DS)
        for wT, bf_list, f8_list in ((wqT, wq_s, wq8_s), (wkT, wk_s, wk8_s)):
            wtile = wT[:, sl].reshape(KT, 128, HS, 128)
            bf_list.append(np.ascontiguousarray(
                (wtile[KA:] * SWB).transpose(1, 2, 0, 3)).astype(bf16))
            f8_list.append(np.ascontiguousarray(
                (wtile[: max(KA, 1)] * SW8).transpose(1, 2, 0, 3)).astype(f8))
        wv_s.append(np.ascontiguousarray(
            wvT[:, sl].reshape(KT, 128, DS).transpose(1, 0, 2)).astype(bf16))
        wo_s.append(np.ascontiguousarray(
            woT[sl].reshape(HS, 128, DM).transpose(1, 0, 2)).astype(bf16))

    inv = 1.0 / THETA ** (np.arange(0, HEAD_DIM, 2, dtype=np.float32) / HEAD_DIM)
    fr = np.outer(inv, np.arange(t_len, dtype=np.float32))  # [64, T]
    cosT = np.cos(fr).astype(np.float32)
    sinT = np.sin(fr).astype(np.float32)
    ccat = np.concatenate([cosT, cosT], axis=0).astype(bf16)   # [128, T]
    scat = np.concatenate([-sinT, sinT], axis=0).astype(bf16)  # [128, T]

    r = np.arange(128)[:, None]
    qc = np.arange(CH)[None, :]
    mask = np.stack([(r <= qc), (128 + r <= qc)], axis=1).astype(bf16)  # [128,2,256]

    in_maps = []
    for core in range(8):
        b, hg = core // 4, core % 4
        in_maps.append({
            "xt": xts[b], "xt8": xt8s[b], "wq": wq_s[hg], "wk": wk_s[hg],
            "wq8": wq8_s[hg], "wk8": wk8_s[hg], "wv": wv_s[hg],
            "wo": wo_s[hg], "ccat": ccat, "scat": scat, "mask": mask,
        })
    return in_maps


def kernel(x, Wq, Wk, Wv, Wo):
    global _NC, LAST_EXEC_NS
    from concourse.bass_utils import run_bass_kernel_spmd

    profile = bool(os.environ.get("KERNEL_PROFILE"))
    if profile:
        try:
            import hook_util
            hook_util.install()
            hook_util.patch_upload()
        except ImportError:
            profile = False

    in_maps = _prep_inputs(x, Wq, Wk, Wv, Wo)
    if _NC is None:
        _NC = _build_nc()

    kwargs = {}
    if profile:
        kwargs["tmpdir"] = os.environ.get("KERNEL_TRACE_DIR") or None
    res = run_bass_kernel_spmd(
        _NC, in_maps, core_ids=list(range(8)), trace=profile, **kwargs
    )
    LAST_EXEC_NS = res.exec_time_ns

    out = np.zeros((B, T, DM), dtype=np.float32)
    for core in range(8):
        out[core // 4] += res.results[core]["y"]
    return out


# revision 10
# speedup vs baseline: 1.0937x; 1.0937x over previous
"""Chunked sliding-window attention (B=2, T=8192, H=16, Dh=128, W=256) on 8
Trainium2 NeuronCores.

Sharding: 8 cores = 2 (batch) x 4 (head groups of 4 heads). Each core computes
q/k/v projections for its 512-wide slice of the 2048 projection dims, RoPE,
chunked attention for its 4 heads, and a partial output projection over its
512 rows of Wo^T. The host sums the 4 partial outputs per batch element.

Device layouts (host-prepared):
  xt   [128, 16, T]   x^T tiles: xt[p, kt, t] = x[b, t, kt*128+p]        (bf16)
  xt8  [128, KA, T]   16*x^T tiles for kt<KA                             (e4m3)
  wq/wk[128, HS, KT-KA, 128] 1024*(Wq_perm)^T bf16 k-tiles >= KA         (bf16)
  wq8/wk8[128, HS, KA, 128]  64*(Wq_perm)^T fp8 k-tiles < KA             (e4m3)
  wv   [128, 16, 512] Wv^T slice (unpermuted)                            (bf16)
  wo   [128, 4, 2048] Wo^T rows for this core's 512 dims                 (bf16)
  ccat [128, T]       [cos; cos] rope table (freq idx on partitions)     (bf16)
  scat [128, T]       [-sin; sin]                                        (bf16)
  mask [128, 2, 256]  transposed causal 0/1 masks for own-chunk kv tiles (bf16)

The rope row-permutation maps interleaved (re,im) pairs to split layout
(re block rows 0..63, im rows 64..127 per head); applied identically to q and
k it leaves scores invariant, and makes rope unit-stride on chip.

Attention is computed in transposed-score layout [kv, q]: masking is a 0/1
multiply after exp; the softmax denominator (a cross-partition sum of the
pre-summed exp tiles) runs on GPSIMD partition_all_reduce, off the PE.

Mixed precision: KA of the 16 contraction tiles of the q/k projections run
as plain e4m3 DoubleRow matmuls (2 k-tiles per pass, ~1.9x the bf16 rate).
Scale frames are uniform powers of two - fp8 x at 16x, fp8 W at 64x, bf16 W
at 1024x - so fp8 and bf16 products accumulate in one PSUM group at 1024x,
folded back in the exp scale. Measured end-to-end rel_max stays ~1.3e-2 at
KA=6 (error scales as sqrt(KA/16) of the all-fp8 4.6e-2). v/o projections
and attention stay bf16: each would add its own quadrature error term and
v feeds the output directly.

Performance notes (measured via NTFF traces; PE busy ~96%):
- PSUM (8 banks) fully budgeted: 2 banks q/k/v proj, 2 o-proj, 2 packed
  score pairs, 2 ou tiles. o-projection is deferred one block so it never
  heads the PE queue while wo / next xt are still in flight.
- The softmax denominator pre-sums the 4 exp tiles on DVE, then ONE gpsimd
  partition_all_reduce per (head, chunk) replaces the ones-matmul (PE is
  the bottleneck; rc is consumed a block later so the ~1.7us PAR latency
  hides).
- Own-chunk kv tile 1 is causally dead for q cols 0..127: scores, exp and
  the AV matmul all run on the live half only (AV order 0,1,3,2 keeps the
  accumulation group's stop flag on a full-width matmul).
- Const DMAs: wq/wk/xt stream on the sync queue in first-PE-use order;
  wv/mask/wo stream concurrently on the scalar engine's DMA queue.
- fp8 everywhere was evaluated and rejected: e4m3 projections give
  rel_max ~4.6e-2 vs the 2e-2 budget; per-projection plain fp8 is 2.2e-2+.
  3-term hi/lo splits are accurate but slower than bf16 on real HW
  (DoubleRow measures 1.9x bf16, so 3 split terms cost 1.5x).
"""

import os

import numpy as np
import ml_dtypes

N_HEAD = 16
HEAD_DIM = 128
WINDOW = 256
THETA = 10000.0
B = 2
T = 8192
DM = 2048
KT = DM // 128      # 16 contraction tiles
KA = 6              # contraction tiles in fp8 for q/k projections (even)
HS = 4              # heads per core
DS = HS * HEAD_DIM  # 512 projection dims per core
BLK = 512           # tokens per pipeline block (2 chunks)
CH = WINDOW         # 256
SCALE = float(HEAD_DIM) ** -0.5
SX8, SW8, SWB = 16.0, 64.0, 1024.0  # fp8 x / fp8 W / bf16 W scale frames

LAST_EXEC_NS = None
_NC = None

bf16 = ml_dtypes.bfloat16
f8 = ml_dtypes.float8_e4m3


def _build_nc(t_len=T):
    from contextlib import ExitStack

    import concourse.tile as tile
    from concourse import bacc, mybir
    from concourse.bass_isa import ReduceOp

    del ReduceOp  # dn stays on the PE: gpsimd partition_all_reduce measured
    # ~2us latency, and the esum->PAR->recip->ot chain holds the ou PSUM
    # tiles long enough to stall the AV matmuls on PSUM WAR (~110us lost).
    fp32 = mybir.dt.float32
    b16 = mybir.dt.bfloat16
    e4 = mybir.dt.float8e4
    DR = mybir.MatmulPerfMode.DoubleRow

    nb = t_len // BLK
    nc = bacc.Bacc("TRN2", target_bir_lowering=False, debug=False)

    KB = KT - KA  # bf16 k-tiles
    xt = nc.dram_tensor(
        "xt", [nb, 128, KT, BLK], b16, kind="ExternalInput"
    ).ap()
    xt8 = nc.dram_tensor(
        "xt8", [nb, 128, max(KA, 1), BLK], e4, kind="ExternalInput"
    ).ap()
    # head-major so the prologue can stream exactly the head the PE needs next
    wq = nc.dram_tensor("wq", [128, HS, KB, 128], b16, kind="ExternalInput").ap()
    wk = nc.dram_tensor("wk", [128, HS, KB, 128], b16, kind="ExternalInput").ap()
    wq8 = nc.dram_tensor("wq8", [128, HS, max(KA, 1), 128], e4, kind="ExternalInput").ap()
    wk8 = nc.dram_tensor("wk8", [128, HS, max(KA, 1), 128], e4, kind="ExternalInput").ap()
    wv = nc.dram_tensor("wv", [128, KT, DS], b16, kind="ExternalInput").ap()
    wo = nc.dram_tensor("wo", [128, HS, DM], b16, kind="ExternalInput").ap()
    ccat = nc.dram_tensor("ccat", [128, t_len], b16, kind="ExternalInput").ap()
    scat = nc.dram_tensor("scat", [128, t_len], b16, kind="ExternalInput").ap()
    mask = nc.dram_tensor("mask", [128, 2, CH], b16, kind="ExternalInput").ap()
    y = nc.dram_tensor("y", [t_len, DM], fp32, kind="ExternalOutput").ap()

    Exp = mybir.ActivationFunctionType.Exp
    ESCALE = SCALE / float(SX8 * SW8) ** 2  # scores carry (16*64)^2

    with tile.TileContext(nc) as tc, ExitStack() as ctx:
        const = ctx.enter_context(tc.tile_pool(name="const", bufs=1))
        xt_p = ctx.enter_context(tc.tile_pool(name="xtp", bufs=2))
        xt8_p = ctx.enter_context(tc.tile_pool(name="xt8p", bufs=2))
        raw_p = ctx.enter_context(tc.tile_pool(name="rawp", bufs=3))
        swp_p = ctx.enter_context(tc.tile_pool(name="swpp", bufs=3))
        tmp_p = ctx.enter_context(tc.tile_pool(name="tmpp", bufs=3))
        qr_p = ctx.enter_context(tc.tile_pool(name="qrp", bufs=8))
        kr_p = ctx.enter_context(tc.tile_pool(name="krp", bufs=10))
        v_p = ctx.enter_context(tc.tile_pool(name="vp", bufs=10))
        e_p = ctx.enter_context(tc.tile_pool(name="ep", bufs=12))
        rc_p = ctx.enter_context(tc.tile_pool(name="rcp", bufs=4))
        ot_p = ctx.enter_context(tc.tile_pool(name="otp", bufs=24))
        y_p = ctx.enter_context(tc.tile_pool(name="yp", bufs=2))
        tab_p = ctx.enter_context(tc.tile_pool(name="tabp", bufs=3))
        # PSUM is 8 banks x [128,512] fp32; bufs are bank-granular.
        ps_big = ctx.enter_context(tc.tile_pool(name="psbig", bufs=2, space="PSUM"))
        ps_op = ctx.enter_context(tc.tile_pool(name="psop", bufs=2, space="PSUM"))
        ps_st = ctx.enter_context(tc.tile_pool(name="psst", bufs=2, space="PSUM"))
        ps_do = ctx.enter_context(tc.tile_pool(name="psdo", bufs=2, space="PSUM"))

        wq_sb = const.tile([128, HS, KB, 128], b16)
        wk_sb = const.tile([128, HS, KB, 128], b16)
        wq8_sb = const.tile([128, HS, max(KA, 1), 128], e4)
        wk8_sb = const.tile([128, HS, max(KA, 1), 128], e4)
        wv_sb = const.tile([128, KT, DS], b16)
        wo_sb = const.tile([128, HS, DM], b16)
        mask_sb = const.tile([128, 2, CH], b16)
        ones_sb = const.tile([128, 128], b16)
        nc.vector.memset(ones_sb, 1.0)

        def fetch_tables(t0):
            cc = tab_p.tile([128, BLK], b16, tag="cc")
            nc.sync.dma_start(cc, ccat[:, t0 : t0 + BLK])
            sc = tab_p.tile([128, BLK], b16, tag="sc")
            nc.sync.dma_start(sc, scat[:, t0 : t0 + BLK])
            return cc, sc

        def fetch_xts(blk):
            xsb = xt_p.tile([128, KT, BLK], b16, tag="xt")
            nc.sync.dma_start(xsb, xt[blk])
            if KA > 0:
                x8 = xt8_p.tile([128, KA, BLK], e4, tag="xt8")
                nc.sync.dma_start(x8, xt8[blk][:, 0:KA, :])
            else:
                x8 = None
            return xsb, x8

        # Single sync queue, pieces ordered by first PE use (the DMA queues
        # share HBM bandwidth, so a second queue racing big low-priority
        # loads ahead just starves the critical q/k pieces). q h0's fp8 DR
        # prefix needs wq8/xt8 first, then its bf16 wq/xt tiles; wo and
        # block 1's xt/tables are deferred behind block 0's rope swaps.
        xt_first = xt_p.tile([128, KT, BLK], b16, tag="xt")
        if KA > 0:
            xt8_first = xt8_p.tile([128, KA, BLK], e4, tag="xt8")
            nc.sync.dma_start(wq8_sb[:, 0], wq8[:, 0])
            nc.sync.dma_start(xt8_first, xt8[0][:, 0:KA, :])
        else:
            xt8_first = None
        nc.sync.dma_start(wq_sb[:, 0, 0:2, :], wq[:, 0, 0:2, :])
        nc.sync.dma_start(xt_first[:, KA : KA + 4, :], xt[0][:, KA : KA + 4, :])
        nc.sync.dma_start(wq_sb[:, 0, 2:KB, :], wq[:, 0, 2:KB, :])
        nc.sync.dma_start(xt_first[:, KA + 4 : KT, :], xt[0][:, KA + 4 : KT, :])
        if KA > 0:
            nc.sync.dma_start(wk8_sb[:, 0], wk8[:, 0])
        nc.sync.dma_start(wk_sb[:, 0], wk[:, 0])
        tab_first = fetch_tables(0)
        for h in (1, 2):
            if KA > 0:
                nc.sync.dma_start(wq8_sb[:, h], wq8[:, h])
                nc.sync.dma_start(wk8_sb[:, h], wk8[:, h])
            nc.sync.dma_start(wq_sb[:, h], wq[:, h])
            nc.sync.dma_start(wk_sb[:, h], wk[:, h])
            if h == 1:
                nc.sync.dma_start(mask_sb, mask)
        nc.sync.dma_start(wv_sb[:, 0:8, :], wv[:, 0:8, :])
        nc.sync.dma_start(xt_first[:, 0:KA, :], xt[0][:, 0:KA, :])
        if KA > 0:
            nc.sync.dma_start(wq8_sb[:, 3], wq8[:, 3])
            nc.sync.dma_start(wk8_sb[:, 3], wk8[:, 3])
        nc.sync.dma_start(wq_sb[:, 3], wq[:, 3])
        nc.sync.dma_start(wk_sb[:, 3], wk[:, 3])
        nc.sync.dma_start(wv_sb[:, 8:KT, :], wv[:, 8:KT, :])
        # tab2/xt2/wo deferred behind blk0's rope swap DMAs on the sync queue
        tab_second = [None]
        xt_second = [None]

        def emit_deferred_consts():
            if nb > 1:
                tab_second[0] = fetch_tables(BLK)
                xt_second[0] = fetch_xts(1)
            for h in range(HS):
                nc.sync.dma_start(wo_sb[:, h, :], wo[:, h, :])

        prev_k = [None] * HS
        prev_v = [None, None]
        pend_ot = None
        for blk in range(nb):
            t0 = blk * BLK
            if blk == 0:
                xt_sb, xt8_sb = xt_first, xt8_first
            elif blk == 1 and xt_second[0] is not None:
                xt_sb, xt8_sb = xt_second[0]
            else:
                xt_sb, xt8_sb = fetch_xts(blk)

            if blk == 0:
                c_sl, s_sl = tab_first
            elif blk == 1 and tab_second[0] is not None:
                c_sl, s_sl = tab_second[0]
            else:
                c_sl, s_sl = fetch_tables(t0)
            cur_q = []
            cur_k = []
            for h in range(HS):
                for w_sb, w8_sb, dst in (
                    (wq_sb, wq8_sb, cur_q),
                    (wk_sb, wk8_sb, cur_k),
                ):
                    ps = ps_big.tile([128, BLK], fp32, tag="psbig")
                    for j in range(KA // 2):
                        nc.tensor.matmul(
                            ps,
                            lhsT=w8_sb[:, h, 2 * j : 2 * j + 2, :],
                            rhs=xt8_sb[:, 2 * j : 2 * j + 2, :],
                            start=(j == 0),
                            stop=False,
                            perf_mode=DR,
                        )
                    for k in range(KB):
                        nc.tensor.matmul(
                            ps,
                            lhsT=w_sb[:, h, k, :],
                            rhs=xt_sb[:, KA + k, :],
                            start=(KA == 0 and k == 0),
                            stop=(k == KB - 1),
                        )
                    raw = raw_p.tile([128, BLK], b16, tag="raw")
                    nc.scalar.copy(raw, ps)
                    # swap the (re, im) halves via SBUF->SBUF DMA (DVE lanes
                    # cannot cross partitions)
                    swp = swp_p.tile([128, BLK], b16, tag="swp")
                    nc.sync.dma_start(swp[0:64, :], raw[64:128, :])
                    nc.sync.dma_start(swp[64:128, :], raw[0:64, :])
                    t1 = tmp_p.tile([128, BLK], b16, tag="t1")
                    nc.vector.tensor_mul(t1, raw, c_sl)
                    t2 = tmp_p.tile([128, BLK], b16, tag="t2")
                    nc.vector.tensor_mul(t2, swp, s_sl)
                    if dst is cur_q:
                        rot = qr_p.tile([128, BLK], b16, tag="qr")
                    else:
                        rot = kr_p.tile([128, BLK], b16, tag="kr")
                    nc.vector.tensor_add(rot, t1, t2)
                    dst.append(rot)

            cur_v = []
            for tt in range(4):
                ps = ps_big.tile([128, BLK], fp32, tag="psbig")
                # k order starts at KA: block 0's xt[0:KA] piece arrives last
                for kk in range(KT):
                    k = (KA + kk) % KT
                    nc.tensor.matmul(
                        ps,
                        lhsT=xt_sb[:, k, tt * 128 : (tt + 1) * 128],
                        rhs=wv_sb[:, k, :],
                        start=(kk == 0),
                        stop=(kk == KT - 1),
                    )
                vt = v_p.tile([128, DS], b16, tag="v")
                nc.vector.tensor_copy(out=vt, in_=ps)
                cur_v.append(vt)

            ot_tiles = {}
            for ci in range(2):
                c = 2 * blk + ci
                qoff = ci * CH
                js = [2, 3] if c == 0 else [0, 1, 2, 3]
                for h in range(HS):
                    q_sl = cur_q[h][:, qoff : qoff + CH]
                    es = []
                    stp = None
                    for idx, j in enumerate(js):
                        if j < 2:
                            if ci == 1:
                                ksrc = cur_k[h][:, j * 128 : (j + 1) * 128]
                            else:
                                ksrc = prev_k[h][:, CH + j * 128 : CH + (j + 1) * 128]
                        else:
                            ksrc = cur_k[h][:, qoff + (j - 2) * 128 : qoff + (j - 1) * 128]
                        if idx % 2 == 0:
                            stp = ps_st.tile([128, 2 * CH], fp32, tag="st")
                        st = stp[:, (idx % 2) * CH : (idx % 2 + 1) * CH]
                        e = e_p.tile([128, CH], b16, tag="e")
                        if j == 3:
                            # kv rows 128.. of own chunk are causally dead for
                            # q cols 0..127: compute only the live half
                            nc.tensor.matmul(
                                st[:, 128:CH], lhsT=ksrc, rhs=q_sl[:, 128:CH],
                                start=True, stop=True,
                            )
                            nc.gpsimd.memset(e[:, 0:128], 0.0)
                            nc.scalar.activation(
                                e[:, 128:CH], st[:, 128:CH], Exp, scale=ESCALE
                            )
                            nc.vector.tensor_mul(
                                e[:, 128:CH], e[:, 128:CH], mask_sb[:, 1, 128:CH]
                            )
                        else:
                            nc.tensor.matmul(st, lhsT=ksrc, rhs=q_sl, start=True, stop=True)
                            nc.scalar.activation(e, st, Exp, scale=ESCALE)
                            if j == 2:
                                nc.vector.tensor_mul(e, e, mask_sb[:, 0, :])
                        es.append((j, e))
                    # pre-sum the exp tiles on DVE; ONE gpsimd
                    # partition_all_reduce then yields the denominator
                    # broadcast across partitions, keeping it off the PE.
                    acc = es[0][1]
                    for i in range(1, len(es)):
                        if i == len(es) - 1:
                            nxt = e_p.tile([128, CH], b16, tag="esum")
                        else:
                            nxt = tmp_p.tile([128, CH], b16, tag=f"ea{i % 2}")
                        nc.vector.tensor_add(nxt, acc, es[i][1])
                        acc = nxt
                    esum = acc
                    combo = ps_do.tile([128, 2 * CH], fp32, tag="do")
                    dn = combo[:, 0:CH]
                    ou = combo[:, CH : 2 * CH]
                    # AV accumulation order 0,1,3,2: j=3 streams only its
                    # causally-live half, the full-width j=2 carries stop.
                    avs = list(enumerate(es))
                    if len(avs) == 4:
                        avs = [avs[0], avs[1], avs[3], avs[2]]
                    for pos, (i, (j, e)) in enumerate(avs):
                        if j < 2:
                            vsrc = cur_v[j] if ci == 1 else prev_v[j]
                        else:
                            vsrc = cur_v[2 * ci + (j - 2)]
                        vs = vsrc[:, h * 128 : (h + 1) * 128]
                        if j == 3 and pos != len(avs) - 1 and pos != 0:
                            nc.tensor.matmul(
                                ou[:, 128:CH], lhsT=vs, rhs=e[:, 128:CH],
                                start=False, stop=False,
                            )
                        else:
                            nc.tensor.matmul(
                                ou, lhsT=vs, rhs=e,
                                start=(pos == 0), stop=(pos == len(avs) - 1),
                            )
                    nc.tensor.matmul(dn, lhsT=ones_sb, rhs=esum, start=True, stop=True)
                    rc = rc_p.tile([128, CH], fp32, tag="rc")
                    nc.vector.reciprocal_approx_fast(out=rc, in_=dn)
                    ot = ot_p.tile([128, CH], b16, tag="ot")
                    nc.vector.tensor_mul(ot, ou, rc)
                    ot_tiles[(h, ci)] = ot

            # deferred const DMAs (tab2/xt2) go behind blk0's rope-swap
            # DMAs in the serial sync queue, not ahead of them
            if blk == 0:
                emit_deferred_consts()

            def emit_oproj(ot_map, base_t0, final=False):
                for tt in range(4):
                    ci, sub = tt // 2, tt % 2
                    ysb = y_p.tile([128, DM], fp32, tag="y")
                    for ct in range(4):
                        yps = ps_op.tile([128, 512], fp32, tag="psop")
                        for h in range(HS):
                            nc.tensor.matmul(
                                yps,
                                lhsT=ot_map[(h, ci)][:, sub * 128 : (sub + 1) * 128],
                                rhs=wo_sb[:, h, ct * 512 : (ct + 1) * 512],
                                start=(h == 0),
                                stop=(h == HS - 1),
                            )
                        ysl = ysb[:, ct * 512 : (ct + 1) * 512]
                        if final:
                            # tail has nothing to interleave: halve the copy
                            # latency (scalar+vector in parallel) and stream
                            # y out per-ct so the last DMA overlaps compute
                            nc.scalar.copy(ysl[:, 0:256], yps[:, 0:256])
                            nc.vector.tensor_copy(out=ysl[:, 256:512], in_=yps[:, 256:512])
                            nc.scalar.dma_start(
                                y[
                                    base_t0 + tt * 128 : base_t0 + (tt + 1) * 128,
                                    ct * 512 : (ct + 1) * 512,
                                ],
                                ysl,
                            )
                        elif ct % 2 == 0:
                            nc.scalar.copy(ysl, yps)
                        else:
                            nc.vector.tensor_copy(out=ysl, in_=yps)
                    if not final:
                        nc.scalar.dma_start(
                            y[base_t0 + tt * 128 : base_t0 + (tt + 1) * 128, :], ysb
                        )

            # o-projection deferred one block so it never heads the PE queue
            # while wo / next xt are still in flight
            if pend_ot is not None:
                emit_oproj(pend_ot[0], pend_ot[1])
            pend_ot = (ot_tiles, t0)
            if blk == nb - 1:
                emit_oproj(ot_tiles, t0, final=True)
                pend_ot = None

            prev_k = cur_k
            prev_v = cur_v[2:4]

    nc.compile()
    return nc


def _rope_perm():
    perm = np.empty(DM, np.int64)
    for h in range(N_HEAD):
        base = h * HEAD_DIM
        perm[base : base + 64] = base + 2 * np.arange(64)
        perm[base + 64 : base + 128] = base + 2 * np.arange(64) + 1
    return perm


def _prep_inputs(x, Wq, Wk, Wv, Wo, t_len=T):
    """Build per-core in_maps. Cores 0-3: batch 0, head groups 0-3; 4-7: batch 1."""
    x = np.asarray(x, dtype=np.float32)
    Wq = np.asarray(Wq, dtype=np.float32)
    Wk = np.asarray(Wk, dtype=np.float32)
    Wv = np.asarray(Wv, dtype=np.float32)
    Wo = np.asarray(Wo, dtype=np.float32)
    nb_b = x.shape[0]
    KB = KT - KA

    perm = _rope_perm()
    wqT = np.ascontiguousarray(Wq[perm].T).astype(np.float32)  # [K, dout_perm]
    wkT = np.ascontiguousarray(Wk[perm].T).astype(np.float32)
    wvT = np.ascontiguousarray(Wv.T).astype(bf16)
    woT = np.ascontiguousarray(Wo.T).astype(bf16)        # [d, c]

    # xt[blk, p, kt, t_in_blk] = x[b, blk*BLK + t, kt*128+p] - block-major so
    # each block's slab is one fully-contiguous DMA read per partition
    nblk = t_len // BLK
    xts, xt8s = [], []
    for b in range(nb_b):
        xT = x[b].T.reshape(KT, 128, nblk, BLK)
        xts.append(np.ascontiguousarray(xT.transpose(2, 1, 0, 3)).astype(bf16))
        x8 = (xT[: max(KA, 1)] * SX8).transpose(2, 1, 0, 3)
        xt8s.append(np.ascontiguousarray(x8).astype(f8))

    wq_s, wk_s, wv_s, wo_s, wq8_s, wk8_s = [], [], [], [], [], []
    for hg in range(4):
        sl = slice(hg * DS, (hg + 1) * DS)
        for wT, bf_list, f8_list in ((wqT, wq_s, wq8_s), (wkT, wk_s, wk8_s)):
            wtile = wT[:, sl].reshape(KT, 128, HS, 128)
            bf_list.append(np.ascontiguousarray(
                (wtile[KA:] * SWB).transpose(1, 2, 0, 3)).astype(bf16))
            f8_list.append(np.ascontiguousarray(
                (wtile[: max(KA, 1)] * SW8).transpose(1, 2, 0, 3)).astype(f8))
        wv_s.append(np.ascontiguousarray(
            wvT[:, sl].reshape(KT, 128, DS).transpose(1, 0, 2)).astype(bf16))
        wo_s.append(np.ascontiguousarray(
            woT[sl].reshape(HS, 128, DM).transpose(1, 0, 2)).astype(bf16))

    inv = 1.0 / THETA ** (np.arange(0, HEAD_DIM, 2, dtype=np.float32) / HEAD_DIM)
    fr = np.outer(inv, np.arange(t_len, dtype=np.float32))  # [64, T]
    cosT = np.cos(fr).astype(np.float32)
    sinT = np.sin(fr).astype(np.float32)
    ccat = np.concatenate([cosT, cosT], axis=0).astype(bf16)   # [128, T]
    scat = np.concatenate([-sinT, sinT], axis=0).astype(bf16)  # [128, T]

    r = np.arange(128)[:, None]
    qc = np.arange(CH)[None, :]
    mask = np.stack([(r <= qc), (128 + r <= qc)], axis=1).astype(bf16)  # [128,2,256]

    in_maps = []
    for core in range(8):
        b, hg = core // 4, core % 4
        in_maps.append({
            "xt": xts[b], "xt8": xt8s[b], "wq": wq_s[hg], "wk": wk_s[hg],
            "wq8": wq8_s[hg], "wk8": wk8_s[hg], "wv": wv_s[hg],
            "wo": wo_s[hg], "ccat": ccat, "scat": scat, "mask": mask,
        })
    return in_maps


def kernel(x, Wq, Wk, Wv, Wo):
    global _NC, LAST_EXEC_NS
    from concourse.bass_utils import run_bass_kernel_spmd

    profile = bool(os.environ.get("KERNEL_PROFILE"))
    if profile:
        try:
            import hook_util
            hook_util.install()
            hook_util.patch_upload()
        except ImportError:
            profile = False

    in_maps = _prep_inputs(x, Wq, Wk, Wv, Wo)
    if _NC is None:
        _NC = _build_nc()

    kwargs = {}
    if profile:
        kwargs["tmpdir"] = os.environ.get("KERNEL_TRACE_DIR") or None
    res = run_bass_kernel_spmd(
        _NC, in_maps, core_ids=list(range(8)), trace=profile, **kwargs
    )
    LAST_EXEC_NS = res.exec_time_ns

    out = np.zeros((B, T, DM), dtype=np.float32)
    for core in range(8):
        out[core // 4] += res.results[core]["y"]
    return out


# revision 13
# speedup vs baseline: 1.1139x; 1.0185x over previous
"""Chunked sliding-window attention (B=2, T=8192, H=16, Dh=128, W=256) on 8
Trainium2 NeuronCores.

Sharding: 8 cores = 2 (batch) x 4 (head groups of 4 heads). Each core computes
q/k/v projections for its 512-wide slice of the 2048 projection dims, RoPE,
chunked attention for its 4 heads, and a partial output projection over its
512 rows of Wo^T. The host sums the 4 partial outputs per batch element.

Device layouts (host-prepared):
  xt   [128, 16, T]   x^T tiles: xt[p, kt, t] = x[b, t, kt*128+p]        (bf16)
  xt8  [128, KA, T]   16*x^T tiles for kt<KA                             (e4m3)
  wq/wk[128, HS, KT-KA, 128] 1024*(Wq_perm)^T bf16 k-tiles >= KA         (bf16)
  wq8/wk8[128, HS, KA, 128]  64*(Wq_perm)^T fp8 k-tiles < KA             (e4m3)
  wv   [128, 16, 512] Wv^T slice (unpermuted)                            (bf16)
  wo   [128, 4, 2048] Wo^T rows for this core's 512 dims                 (bf16)
  ccat [128, T]       [cos; cos] rope table (freq idx on partitions)     (bf16)
  scat [128, T]       [-sin; sin]                                        (bf16)
  mask [128, 2, 256]  transposed causal 0/1 masks for own-chunk kv tiles (bf16)

The rope row-permutation maps interleaved (re,im) pairs to split layout
(re block rows 0..63, im rows 64..127 per head); applied identically to q and
k it leaves scores invariant, and makes rope unit-stride on chip.

Attention is computed in transposed-score layout [kv, q]: masking is a 0/1
multiply after exp; the softmax denominator (a cross-partition sum of the
pre-summed exp tiles) runs on GPSIMD partition_all_reduce, off the PE.

Mixed precision: KA of the 16 contraction tiles of the q/k projections run
as plain e4m3 DoubleRow matmuls (2 k-tiles per pass, ~1.9x the bf16 rate).
Scale frames are uniform powers of two - fp8 x at 16x, fp8 W at 64x, bf16 W
at 1024x - so fp8 and bf16 products accumulate in one PSUM group at 1024x,
folded back in the exp scale. Measured end-to-end rel_max stays ~1.3e-2 at
KA=6 (error scales as sqrt(KA/16) of the all-fp8 4.6e-2). v/o projections
and attention stay bf16: each would add its own quadrature error term and
v feeds the output directly.

Performance notes (measured via NTFF traces; PE busy ~96%):
- PSUM (8 banks) fully budgeted: 2 banks q/k/v proj, 2 o-proj, 2 packed
  score pairs, 2 ou tiles. o-projection is deferred one block so it never
  heads the PE queue while wo / next xt are still in flight.
- The softmax denominator pre-sums the 4 exp tiles on DVE, then ONE gpsimd
  partition_all_reduce per (head, chunk) replaces the ones-matmul (PE is
  the bottleneck; rc is consumed a block later so the ~1.7us PAR latency
  hides).
- Own-chunk kv tile 1 is causally dead for q cols 0..127: scores, exp and
  the AV matmul all run on the live half only (AV order 0,1,3,2 keeps the
  accumulation group's stop flag on a full-width matmul).
- Const DMAs: wq/wk/xt stream on the sync queue in first-PE-use order;
  wv/mask/wo stream concurrently on the scalar engine's DMA queue.
- fp8 everywhere was evaluated and rejected: e4m3 projections give
  rel_max ~4.6e-2 vs the 2e-2 budget; per-projection plain fp8 is 2.2e-2+.
  3-term hi/lo splits are accurate but slower than bf16 on real HW
  (DoubleRow measures 1.9x bf16, so 3 split terms cost 1.5x).
"""

import os

import numpy as np
import ml_dtypes

N_HEAD = 16
HEAD_DIM = 128
WINDOW = 256
THETA = 10000.0
B = 2
T = 8192
DM = 2048
KT = DM // 128      # 16 contraction tiles
KA = 6              # contraction tiles in fp8 for q/k projections (even)
HS = 4              # heads per core
DS = HS * HEAD_DIM  # 512 projection dims per core
BLK = 512           # tokens per pipeline block (2 chunks)
CH = WINDOW         # 256
SCALE = float(HEAD_DIM) ** -0.5
SX8, SW8, SWB = 16.0, 64.0, 1024.0  # fp8 x / fp8 W / bf16 W scale frames

LAST_EXEC_NS = None
_NC = None

bf16 = ml_dtypes.bfloat16
f8 = ml_dtypes.float8_e4m3


def _build_nc(t_len=T):
    from contextlib import ExitStack

    import concourse.tile as tile
    from concourse import bacc, mybir
    from concourse.bass_isa import ReduceOp

    del ReduceOp  # dn stays on the PE: gpsimd partition_all_reduce measured
    # ~2us latency, and the esum->PAR->recip->ot chain holds the ou PSUM
    # tiles long enough to stall the AV matmuls on PSUM WAR (~110us lost).
    fp32 = mybir.dt.float32
    b16 = mybir.dt.bfloat16
    e4 = mybir.dt.float8e4
    DR = mybir.MatmulPerfMode.DoubleRow

    nb = t_len // BLK
    nc = bacc.Bacc("TRN2", target_bir_lowering=False, debug=False)

    KB = KT - KA  # bf16 k-tiles
    xt = nc.dram_tensor(
        "xt", [nb, 128, KT, BLK], b16, kind="ExternalInput"
    ).ap()
    xt8 = nc.dram_tensor(
        "xt8", [nb, 128, max(KA, 1), BLK], e4, kind="ExternalInput"
    ).ap()
    # head-major so the prologue can stream exactly the head the PE needs next
    wq = nc.dram_tensor("wq", [128, HS, KB, 128], b16, kind="ExternalInput").ap()
    wk = nc.dram_tensor("wk", [128, HS, KB, 128], b16, kind="ExternalInput").ap()
    wq8 = nc.dram_tensor("wq8", [128, HS, max(KA, 1), 128], e4, kind="ExternalInput").ap()
    wk8 = nc.dram_tensor("wk8", [128, HS, max(KA, 1), 128], e4, kind="ExternalInput").ap()
    wv = nc.dram_tensor("wv", [128, KT, DS], b16, kind="ExternalInput").ap()
    wo = nc.dram_tensor("wo", [128, HS, DM], b16, kind="ExternalInput").ap()
    ccat = nc.dram_tensor("ccat", [128, t_len], b16, kind="ExternalInput").ap()
    scat = nc.dram_tensor("scat", [128, t_len], b16, kind="ExternalInput").ap()
    mask = nc.dram_tensor("mask", [128, 2, CH], b16, kind="ExternalInput").ap()
    y = nc.dram_tensor("y", [t_len, DM], fp32, kind="ExternalOutput").ap()

    Exp = mybir.ActivationFunctionType.Exp
    ESCALE = SCALE / float(SX8 * SW8) ** 2  # scores carry (16*64)^2

    with tile.TileContext(nc) as tc, ExitStack() as ctx:
        const = ctx.enter_context(tc.tile_pool(name="const", bufs=1))
        xt_p = ctx.enter_context(tc.tile_pool(name="xtp", bufs=2))
        xt8_p = ctx.enter_context(tc.tile_pool(name="xt8p", bufs=2))
        raw_p = ctx.enter_context(tc.tile_pool(name="rawp", bufs=3))
        swp_p = ctx.enter_context(tc.tile_pool(name="swpp", bufs=3))
        tmp_p = ctx.enter_context(tc.tile_pool(name="tmpp", bufs=3))
        qr_p = ctx.enter_context(tc.tile_pool(name="qrp", bufs=8))
        kr_p = ctx.enter_context(tc.tile_pool(name="krp", bufs=10))
        v_p = ctx.enter_context(tc.tile_pool(name="vp", bufs=10))
        e_p = ctx.enter_context(tc.tile_pool(name="ep", bufs=12))
        rc_p = ctx.enter_context(tc.tile_pool(name="rcp", bufs=4))
        ot_p = ctx.enter_context(tc.tile_pool(name="otp", bufs=24))
        y_p = ctx.enter_context(tc.tile_pool(name="yp", bufs=2))
        tab_p = ctx.enter_context(tc.tile_pool(name="tabp", bufs=3))
        # PSUM is 8 banks x [128,512] fp32; bufs are bank-granular.
        ps_big = ctx.enter_context(tc.tile_pool(name="psbig", bufs=2, space="PSUM"))
        ps_op = ctx.enter_context(tc.tile_pool(name="psop", bufs=2, space="PSUM"))
        ps_st = ctx.enter_context(tc.tile_pool(name="psst", bufs=2, space="PSUM"))
        ps_do = ctx.enter_context(tc.tile_pool(name="psdo", bufs=2, space="PSUM"))

        wq_sb = const.tile([128, HS, KB, 128], b16)
        wk_sb = const.tile([128, HS, KB, 128], b16)
        wq8_sb = const.tile([128, HS, max(KA, 1), 128], e4)
        wk8_sb = const.tile([128, HS, max(KA, 1), 128], e4)
        wv_sb = const.tile([128, KT, DS], b16)
        wo_sb = const.tile([128, HS, DM], b16)
        mask_sb = const.tile([128, 2, CH], b16)
        ones_sb = const.tile([128, 128], b16)
        nc.vector.memset(ones_sb, 1.0)

        def fetch_tables(t0):
            cc = tab_p.tile([128, BLK], b16, tag="cc")
            nc.sync.dma_start(cc, ccat[:, t0 : t0 + BLK])
            sc = tab_p.tile([128, BLK], b16, tag="sc")
            nc.sync.dma_start(sc, scat[:, t0 : t0 + BLK])
            return cc, sc

        def fetch_xts(blk):
            xsb = xt_p.tile([128, KT, BLK], b16, tag="xt")
            nc.sync.dma_start(xsb, xt[blk])
            if KA > 0:
                x8 = xt8_p.tile([128, KA, BLK], e4, tag="xt8")
                nc.sync.dma_start(x8, xt8[blk][:, 0:KA, :])
            else:
                x8 = None
            return xsb, x8

        # Single sync queue, pieces ordered by first PE use (the DMA queues
        # share HBM bandwidth, so a second queue racing big low-priority
        # loads ahead just starves the critical q/k pieces). q h0's fp8 DR
        # prefix needs wq8/xt8 first, then its bf16 wq/xt tiles; wo and
        # block 1's xt/tables are deferred behind block 0's rope swaps.
        xt_first = xt_p.tile([128, KT, BLK], b16, tag="xt")
        if KA > 0:
            xt8_first = xt8_p.tile([128, KA, BLK], e4, tag="xt8")
            nc.sync.dma_start(wq8_sb[:, 0], wq8[:, 0])
            # split so the first DR matmul only waits on its own k-pair
            nc.sync.dma_start(xt8_first[:, 0:2, :], xt8[0][:, 0:2, :])
            if KA > 2:
                nc.sync.dma_start(xt8_first[:, 2:KA, :], xt8[0][:, 2:KA, :])
        else:
            xt8_first = None
        nc.sync.dma_start(wq_sb[:, 0, 0:2, :], wq[:, 0, 0:2, :])
        nc.sync.dma_start(xt_first[:, KA : KA + 4, :], xt[0][:, KA : KA + 4, :])
        nc.sync.dma_start(wq_sb[:, 0, 2:KB, :], wq[:, 0, 2:KB, :])
        nc.sync.dma_start(xt_first[:, KA + 4 : KT, :], xt[0][:, KA + 4 : KT, :])
        if KA > 0:
            nc.sync.dma_start(wk8_sb[:, 0], wk8[:, 0])
        nc.sync.dma_start(wk_sb[:, 0], wk[:, 0])
        tab_first = fetch_tables(0)
        for h in (1, 2, 3):
            if KA > 0:
                nc.sync.dma_start(wq8_sb[:, h], wq8[:, h])
                nc.sync.dma_start(wk8_sb[:, h], wk8[:, h])
            nc.sync.dma_start(wq_sb[:, h], wq[:, h])
            nc.sync.dma_start(wk_sb[:, h], wk[:, h])
            if h == 1:
                nc.sync.dma_start(mask_sb, mask)
        nc.sync.dma_start(wv_sb[:, 0:8, :], wv[:, 0:8, :])
        nc.sync.dma_start(xt_first[:, 0:KA, :], xt[0][:, 0:KA, :])
        nc.sync.dma_start(wv_sb[:, 8:KT, :], wv[:, 8:KT, :])
        # tab2/xt2/wo deferred behind blk0's rope swap DMAs on the sync queue
        tab_second = [None]
        xt_second = [None]

        def emit_deferred_consts():
            if nb > 1:
                tab_second[0] = fetch_tables(BLK)
                xt_second[0] = fetch_xts(1)
            for h in range(HS):
                nc.sync.dma_start(wo_sb[:, h, :], wo[:, h, :])

        prev_k = [None] * HS
        prev_v = [None, None]
        pend_ot = None
        for blk in range(nb):
            t0 = blk * BLK
            if blk == 0:
                xt_sb, xt8_sb = xt_first, xt8_first
            elif blk == 1 and xt_second[0] is not None:
                xt_sb, xt8_sb = xt_second[0]
            else:
                xt_sb, xt8_sb = fetch_xts(blk)

            if blk == 0:
                c_sl, s_sl = tab_first
            elif blk == 1 and tab_second[0] is not None:
                c_sl, s_sl = tab_second[0]
            else:
                c_sl, s_sl = fetch_tables(t0)
            cur_q = []
            cur_k = []
            for h in range(HS):
                for w_sb, w8_sb, dst in (
                    (wq_sb, wq8_sb, cur_q),
                    (wk_sb, wk8_sb, cur_k),
                ):
                    ps = ps_big.tile([128, BLK], fp32, tag="psbig")
                    for j in range(KA // 2):
                        nc.tensor.matmul(
                            ps,
                            lhsT=w8_sb[:, h, 2 * j : 2 * j + 2, :],
                            rhs=xt8_sb[:, 2 * j : 2 * j + 2, :],
                            start=(j == 0),
                            stop=False,
                            perf_mode=DR,
                        )
                    for k in range(KB):
                        nc.tensor.matmul(
                            ps,
                            lhsT=w_sb[:, h, k, :],
                            rhs=xt_sb[:, KA + k, :],
                            start=(KA == 0 and k == 0),
                            stop=(k == KB - 1),
                        )
                    raw = raw_p.tile([128, BLK], b16, tag="raw")
                    nc.scalar.copy(raw, ps)
                    # swap the (re, im) halves via SBUF->SBUF DMA (DVE lanes
                    # cannot cross partitions)
                    swp = swp_p.tile([128, BLK], b16, tag="swp")
                    nc.sync.dma_start(swp[0:64, :], raw[64:128, :])
                    nc.sync.dma_start(swp[64:128, :], raw[0:64, :])
                    t1 = tmp_p.tile([128, BLK], b16, tag="t1")
                    nc.vector.tensor_mul(t1, raw, c_sl)
                    t2 = tmp_p.tile([128, BLK], b16, tag="t2")
                    nc.vector.tensor_mul(t2, swp, s_sl)
                    if dst is cur_q:
                        rot = qr_p.tile([128, BLK], b16, tag="qr")
                    else:
                        rot = kr_p.tile([128, BLK], b16, tag="kr")
                    nc.vector.tensor_add(rot, t1, t2)
                    dst.append(rot)

            cur_v = []
            for tt in range(4):
                ps = ps_big.tile([128, BLK], fp32, tag="psbig")
                # k order starts at KA: block 0's xt[0:KA] piece arrives last
                for kk in range(KT):
                    k = (KA + kk) % KT
                    nc.tensor.matmul(
                        ps,
                        lhsT=xt_sb[:, k, tt * 128 : (tt + 1) * 128],
                        rhs=wv_sb[:, k, :],
                        start=(kk == 0),
                        stop=(kk == KT - 1),
                    )
                vt = v_p.tile([128, DS], b16, tag="v")
                nc.vector.tensor_copy(out=vt, in_=ps)
                cur_v.append(vt)

            ot_tiles = {}

            def emit_scores(ci, c, h):
                """Scores + exp + mask + esum chain for one (head, chunk)."""
                qoff = ci * CH
                js = [2, 3] if c == 0 else [0, 1, 2, 3]
                q_sl = cur_q[h][:, qoff : qoff + CH]
                es = []
                stp = None
                for idx, j in enumerate(js):
                    if j < 2:
                        if ci == 1:
                            ksrc = cur_k[h][:, j * 128 : (j + 1) * 128]
                        else:
                            ksrc = prev_k[h][:, CH + j * 128 : CH + (j + 1) * 128]
                    else:
                        ksrc = cur_k[h][:, qoff + (j - 2) * 128 : qoff + (j - 1) * 128]
                    if idx % 2 == 0:
                        stp = ps_st.tile([128, 2 * CH], fp32, tag="st")
                    st = stp[:, (idx % 2) * CH : (idx % 2 + 1) * CH]
                    e = e_p.tile([128, CH], b16, tag="e")
                    if j == 3:
                        # kv rows 128.. of own chunk are causally dead for
                        # q cols 0..127: compute only the live half
                        nc.tensor.matmul(
                            st[:, 128:CH], lhsT=ksrc, rhs=q_sl[:, 128:CH],
                            start=True, stop=True,
                        )
                        nc.gpsimd.memset(e[:, 0:128], 0.0)
                        nc.scalar.activation(
                            e[:, 128:CH], st[:, 128:CH], Exp, scale=ESCALE
                        )
                        nc.vector.tensor_mul(
                            e[:, 128:CH], e[:, 128:CH], mask_sb[:, 1, 128:CH]
                        )
                    else:
                        nc.tensor.matmul(st, lhsT=ksrc, rhs=q_sl, start=True, stop=True)
                        nc.scalar.activation(e, st, Exp, scale=ESCALE)
                        if j == 2:
                            nc.vector.tensor_mul(e, e, mask_sb[:, 0, :])
                    es.append((j, e))
                # pre-sum the exp tiles on DVE so the partition-reduce
                # ones-matmul streams once, not len(es) times
                acc = es[0][1]
                for i in range(1, len(es)):
                    if i == len(es) - 1:
                        nxt = e_p.tile([128, CH], b16, tag="esum")
                    else:
                        nxt = tmp_p.tile([128, CH], b16, tag=f"ea{i % 2}")
                    nc.vector.tensor_add(nxt, acc, es[i][1])
                    acc = nxt
                return ci, h, es, acc

            def emit_av_dn(grp):
                """AV matmuls + denominator + normalize for one group."""
                ci, h, es, esum = grp
                combo = ps_do.tile([128, 2 * CH], fp32, tag="do")
                dn = combo[:, 0:CH]
                ou = combo[:, CH : 2 * CH]
                # AV accumulation order 0,1,3,2: j=3 streams only its
                # causally-live half, the full-width j=2 carries stop.
                avs = list(es)
                if len(avs) == 4:
                    avs = [avs[0], avs[1], avs[3], avs[2]]
                for pos, (j, e) in enumerate(avs):
                    if j < 2:
                        vsrc = cur_v[j] if ci == 1 else prev_v[j]
                    else:
                        vsrc = cur_v[2 * ci + (j - 2)]
                    vs = vsrc[:, h * 128 : (h + 1) * 128]
                    if j == 3 and pos != len(avs) - 1 and pos != 0:
                        nc.tensor.matmul(
                            ou[:, 128:CH], lhsT=vs, rhs=e[:, 128:CH],
                            start=False, stop=False,
                        )
                    else:
                        nc.tensor.matmul(
                            ou, lhsT=vs, rhs=e,
                            start=(pos == 0), stop=(pos == len(avs) - 1),
                        )
                nc.tensor.matmul(dn, lhsT=ones_sb, rhs=esum, start=True, stop=True)
                rc = rc_p.tile([128, CH], fp32, tag="rc")
                nc.vector.reciprocal_approx_fast(out=rc, in_=dn)
                ot = ot_p.tile([128, CH], b16, tag="ot")
                nc.vector.tensor_mul(ot, ou, rc)
                ot_tiles[(h, ci)] = ot

            # Software-pipeline the groups: group g's AV/dn matmuls are
            # emitted after group g+1's scores, so the PE has scores work
            # to chew while g's exp/mask/esum chain drains on ACT/DVE.
            pend_grp = None
            for ci in range(2):
                c = 2 * blk + ci
                for h in range(HS):
                    grp = emit_scores(ci, c, h)
                    if pend_grp is not None:
                        emit_av_dn(pend_grp)
                    pend_grp = grp
            emit_av_dn(pend_grp)

            # deferred const DMAs (tab2/xt2) go behind blk0's rope-swap
            # DMAs in the serial sync queue, not ahead of them
            if blk == 0:
                emit_deferred_consts()

            def emit_oproj(ot_map, base_t0, final=False):
                for tt in range(4):
                    ci, sub = tt // 2, tt % 2
                    ysb = y_p.tile([128, DM], fp32, tag="y")
                    for ct in range(4):
                        yps = ps_op.tile([128, 512], fp32, tag="psop")
                        for h in range(HS):
                            nc.tensor.matmul(
                                yps,
                                lhsT=ot_map[(h, ci)][:, sub * 128 : (sub + 1) * 128],
                                rhs=wo_sb[:, h, ct * 512 : (ct + 1) * 512],
                                start=(h == 0),
                                stop=(h == HS - 1),
                            )
                        ysl = ysb[:, ct * 512 : (ct + 1) * 512]
                        if final:
                            # tail has nothing to interleave: halve the copy
                            # latency (scalar+vector in parallel) and stream
                            # y out per-ct so the last DMA overlaps compute
                            nc.scalar.copy(ysl[:, 0:256], yps[:, 0:256])
                            nc.vector.tensor_copy(out=ysl[:, 256:512], in_=yps[:, 256:512])
                            nc.scalar.dma_start(
                                y[
                                    base_t0 + tt * 128 : base_t0 + (tt + 1) * 128,
                                    ct * 512 : (ct + 1) * 512,
                                ],
                                ysl,
                            )
                        elif ct % 2 == 0:
                            nc.scalar.copy(ysl, yps)
                        else:
                            nc.vector.tensor_copy(out=ysl, in_=yps)
                    if not final:
                        nc.scalar.dma_start(
                            y[base_t0 + tt * 128 : base_t0 + (tt + 1) * 128, :], ysb
                        )

            # o-projection deferred one block so it never heads the PE queue
            # while wo / next xt are still in flight
            if pend_ot is not None:
                emit_oproj(pend_ot[0], pend_ot[1])
            pend_ot = (ot_tiles, t0)
            if blk == nb - 1:
                emit_oproj(ot_tiles, t0, final=True)
                pend_ot = None

            prev_k = cur_k
            prev_v = cur_v[2:4]

    nc.compile()
    return nc


def _rope_perm():
    perm = np.empty(DM, np.int64)
    for h in range(N_HEAD):
        base = h * HEAD_DIM
        perm[base : base + 64] = base + 2 * np.arange(64)
        perm[base + 64 : base + 128] = base + 2 * np.arange(64) + 1
    return perm


def _prep_inputs(x, Wq, Wk, Wv, Wo, t_len=T):
    """Build per-core in_maps. Cores 0-3: batch 0, head groups 0-3; 4-7: batch 1."""
    x = np.asarray(x, dtype=np.float32)
    Wq = np.asarray(Wq, dtype=np.float32)
    Wk = np.asarray(Wk, dtype=np.float32)
    Wv = np.asarray(Wv, dtype=np.float32)
    Wo = np.asarray(Wo, dtype=np.float32)
    nb_b = x.shape[0]
    KB = KT - KA

    perm = _rope_perm()
    wqT = np.ascontiguousarray(Wq[perm].T).astype(np.float32)  # [K, dout_perm]
    wkT = np.ascontiguousarray(Wk[perm].T).astype(np.float32)
    wvT = np.ascontiguousarray(Wv.T).astype(bf16)
    woT = np.ascontiguousarray(Wo.T).astype(bf16)        # [d, c]

    # xt[blk, p, kt, t_in_blk] = x[b, blk*BLK + t, kt*128+p] - block-major so
    # each block's slab is one fully-contiguous DMA read per partition
    nblk = t_len // BLK
    xts, xt8s = [], []
    for b in range(nb_b):
        xT = x[b].T.reshape(KT, 128, nblk, BLK)
        xts.append(np.ascontiguousarray(xT.transpose(2, 1, 0, 3)).astype(bf16))
        x8 = (xT[: max(KA, 1)] * SX8).transpose(2, 1, 0, 3)
        xt8s.append(np.ascontiguousarray(x8).astype(f8))

    wq_s, wk_s, wv_s, wo_s, wq8_s, wk8_s = [], [], [], [], [], []
    for hg in range(4):
        sl = slice(hg * DS, (hg + 1) * DS)
        for wT, bf_list, f8_list in ((wqT, wq_s, wq8_s), (wkT, wk_s, wk8_s)):
            wtile = wT[:, sl].reshape(KT, 128, HS, 128)
            bf_list.append(np.ascontiguousarray(
                (wtile[KA:] * SWB).transpose(1, 2, 0, 3)).astype(bf16))
            f8_list.append(np.ascontiguousarray(
                (wtile[: max(KA, 1)] * SW8).transpose(1, 2, 0, 3)).astype(f8))
        wv_s.append(np.ascontiguousarray(
            wvT[:, sl].reshape(KT, 128, DS).transpose(1, 0, 2)).astype(bf16))
        wo_s.append(np.ascontiguousarray(
            woT[sl].reshape(HS, 128, DM).transpose(1, 0, 2)).astype(bf16))

    inv = 1.0 / THETA ** (np.arange(0, HEAD_DIM, 2, dtype=np.float32) / HEAD_DIM)
    fr = np.outer(inv, np.arange(t_len, dtype=np.float32))  # [64, T]
    cosT = np.cos(fr).astype(np.float32)
    sinT = np.sin(fr).astype(np.float32)
    ccat = np.concatenate([cosT, cosT], axis=0).astype(bf16)   # [128, T]
    scat = np.concatenate([-sinT, sinT], axis=0).astype(bf16)  # [128, T]

    r = np.arange(128)[:, None]
    qc = np.arange(CH)[None, :]
    mask = np.stack([(r <= qc), (128 + r <= qc)], axis=1).astype(bf16)  # [128,2,256]

    in_maps = []
    for core in range(8):
        b, hg = core // 4, core % 4
        in_maps.append({
            "xt": xts[b], "xt8": xt8s[b], "wq": wq_s[hg], "wk": wk_s[hg],
            "wq8": wq8_s[hg], "wk8": wk8_s[hg], "wv": wv_s[hg],
            "wo": wo_s[hg], "ccat": ccat, "scat": scat, "mask": mask,
        })
    return in_maps


def kernel(x, Wq, Wk, Wv, Wo):
    global _NC, LAST_EXEC_NS
    from concourse.bass_utils import run_bass_kernel_spmd

    profile = bool(os.environ.get("KERNEL_PROFILE"))
    if profile:
        try:
            import hook_util
            hook_util.install()
            hook_util.patch_upload()
        except ImportError:
            profile = False

    in_maps = _prep_inputs(x, Wq, Wk, Wv, Wo)
    if _NC is None:
        _NC = _build_nc()

    kwargs = {}
    if profile:
        kwargs["tmpdir"] = os.environ.get("KERNEL_TRACE_DIR") or None
    res = run_bass_kernel_spmd(
        _NC, in_maps, core_ids=list(range(8)), trace=profile, **kwargs
    )
    LAST_EXEC_NS = res.exec_time_ns

    out = np.zeros((B, T, DM), dtype=np.float32)
    for core in range(8):
        out[core // 4] += res.results[core]["y"]
    return out
